# revision 1
# baseline (speedup 1.0000x reference)
"""Trainium2 Bass kernel for nn_ClusterLoss (N=4096, D=2048, 8 NeuronCores).

Math (constants ALPHA=6, BETA=2, ANN_R=3, ANN_RR=5, TVAL=1, EPS=1e-5):
  dm = 1 - dist <= 1 < BETA  =>  loss_ap == 0 identically.
  dm < ALPHA always          =>  an_mask == neg (upper-tri & label mismatch).
  loss_an_i = sum_j (5+u_ij) e^(5+u_ij) / (sum_j e^(5+u_ij) + EPS),  u = dist.
Device computes per-row S0 = sum w and S1 = sum u*w with w = e^(u+5) masked;
host does the division, mean, and the annulus term (O(N) work).

Sharding: rows are split into 8 blocks of 512; core c owns the 64-row slice c
of every block ("half-tiles"), pairing blocks (0,1),(2,3),(4,5),(6,7) into 4
fused 128-row m-tiles so the upper-triangular tile skip is load-balanced AND
the program is identical on all cores (SPMD) — only the gathered input data
differs per core.

The [128,512] distance tile comes out of one augmented bf16 matmul:
  lhsT rows 0..2047 = -2*cf_mine.T, then [1, 1, sqh_i, sql_i]
  rhs  rows 0..2047 =    cf_all.T,  then [sqh_j, sql_j, 1, 1]
so PSUM = sq_i + sq_j - 2*cf_i.cf_j exactly (sq split hi+lo in bf16).
A second tiny matmul with +/- one-hot label rows yields (1 - same_label)
directly in PSUM. DVE tensor_mul + reduce_sum do masking and row-sums
(tensor_tensor_reduce faults on this hardware path; plain ops do not).
"""

import sys

sys.path.insert(0, "/opt/trn_rl_repo")

import numpy as np
import ml_dtypes

import concourse.bass as bass
import concourse.mybir as mybir
import concourse.tile as tile
from concourse import bacc
from concourse.bass_utils import run_bass_kernel_spmd

BF16 = ml_dtypes.bfloat16
N, D, NCORES = 4096, 2048, 8
QBLK = 512          # row block per q
HALF = 64           # per-core slice of each q block
KTOT = D + 4        # 2052 augmented K rows
KCH = 17            # ceil(2052/128); padded to 17*128 = 2176 with zeros
KPAD = KCH * 128
NB = 8              # 512-wide n blocks
FT = 4              # fused m-tiles per core

_prog_cache = {}


def _build_program():
    nc = bacc.Bacc("TRN2", target_bir_lowering=False, debug=False,
                   num_devices=NCORES)

    # const AP for the Exp bias (+5.0), registered in the preamble like
    # Bass.__init__ does for 0.0/1.0
    t5 = nc.alloc_sbuf_tensor("const-float32-5.0", [128, 1], mybir.dt.float32)
    nc.gpsimd.memset(t5.ap(), 5.0)
    nc.const_aps.aps[(mybir.dt.float32, 5.0)] = t5.ap()
    nc.all_engine_barrier()

    a_d = nc.dram_tensor("a", [128, NB, KCH, 512], mybir.dt.bfloat16,
                         kind="ExternalInput")
    rm2_d = nc.dram_tensor("rm2", [128, KCH, 512], mybir.dt.bfloat16,
                           kind="ExternalInput")
    oha_d = nc.dram_tensor("oha", [128, N], mybir.dt.bfloat16,
                           kind="ExternalInput")
    ohm_d = nc.dram_tensor("ohm", [128, 512], mybir.dt.bfloat16,
                           kind="ExternalInput")
    mask_d = nc.dram_tensor("masks", [128, NB, 512], mybir.dt.bfloat16,
                            kind="ExternalInput")
    s01_d = nc.dram_tensor("s01", [128, 512], mybir.dt.float32,
                           kind="ExternalOutput")

    fp32 = mybir.dt.float32
    bf16 = mybir.dt.bfloat16

    with tile.TileContext(nc) as tc:
        with (
            tc.tile_pool(name="big", bufs=1) as big,
            tc.tile_pool(name="abuf", bufs=4) as abuf,
            tc.tile_pool(name="acc", bufs=1) as accp,
            tc.tile_pool(name="work", bufs=4) as work,
            tc.tile_pool(name="psum", bufs=3, space="PSUM") as psum,
        ):
            rm2 = big.tile([128, KCH, 512], bf16)
            nc.sync.dma_start(out=rm2[:], in_=rm2_d.ap())
            ohm = big.tile([128, 512], bf16)
            nc.sync.dma_start(out=ohm[:], in_=ohm_d.ap())
            oha = big.tile([128, N], bf16)
            nc.sync.dma_start(out=oha[:], in_=oha_d.ap())
            masks = big.tile([128, NB, 512], bf16)
            nc.sync.dma_start(out=masks[:], in_=mask_d.ap())


            s0col = [accp.tile([128, NB], fp32, tag=f"s0c{f}", name=f"s0c{f}")
                     for f in range(FT)]
            s1col = [accp.tile([128, NB], fp32, tag=f"s1c{f}", name=f"s1c{f}")
                     for f in range(FT)]

            # n-blocks big-to-small so PE stays ahead of the A DMA stream
            for b in range(NB - 1, -1, -1):
                asb = abuf.tile([128, KCH, 512], bf16, tag="asb", name=f"asb{b}")
                nc.sync.dma_start(out=asb[:], in_=a_d.ap()[:, b])
                for f in range(FT):
                    if b < 2 * f:
                        continue  # tile entirely below the diagonal
                    d2 = psum.tile([128, 512], fp32, tag="d2")
                    for k in range(KCH):
                        nc.tensor.matmul(
                            d2[:],
                            rm2[:, k, 128 * f:128 * (f + 1)],
                            asb[:, k],
                            start=(k == 0),
                            stop=(k == KCH - 1),
                        )
                    nm = psum.tile([128, 512], fp32, tag="nm")
                    nc.tensor.matmul(
                        nm[:],
                        ohm[:, 128 * f:128 * (f + 1)],
                        oha[:, 512 * b:512 * (b + 1)],
                        start=True,
                        stop=True,
                    )
                    diag = b <= 2 * f + 1
                    if diag:
                        # only diagonal-adjacent tiles can have d2 <= 0
                        d2c = work.tile([128, 512], fp32, tag="d2c")
                        nc.vector.tensor_scalar_max(d2c[:], d2[:], 1e-12)
                        usrc = d2c
                    else:
                        usrc = d2
                    u = work.tile([128, 512], fp32, tag="u")
                    nc.scalar.activation(u[:], usrc[:],
                                         mybir.ActivationFunctionType.Sqrt)
                    u2 = work.tile([128, 512], fp32, tag="u2")
                    nc.vector.tensor_add(u2[:], u[:], nm[:])
                    if diag:
                        u3 = work.tile([128, 512], fp32, tag="u3")
                        nc.vector.tensor_add(u3[:], u2[:], masks[:, b])
                    else:
                        u3 = u2
                    cb = b - 2 * f
                    e = work.tile([128, 512], bf16, tag="e")
                    nc.scalar.activation(e[:], u3[:],
                                         mybir.ActivationFunctionType.Exp,
                                         bias=5.0, scale=1.0,
                                         accum_out=s0col[f][:, cb:cb + 1])
                    p = work.tile([128, 512], bf16, tag="p")
                    nc.vector.tensor_mul(p[:], u3[:], e[:])
                    nc.vector.reduce_sum(out=s1col[f][:, cb:cb + 1], in_=p[:],
                                         axis=mybir.AxisListType.X)

            s01 = accp.tile([128, 512], fp32)
            nc.scalar.mul(s01[:], s01[:], 0.0)
            for f in range(FT):
                cnt = NB - 2 * f
                nc.vector.reduce_sum(out=s01[:, f:f + 1], in_=s0col[f][:, :cnt],
                                     axis=mybir.AxisListType.X)
                nc.vector.reduce_sum(out=s01[:, FT + f:FT + f + 1],
                                     in_=s1col[f][:, :cnt],
                                     axis=mybir.AxisListType.X)
            nc.sync.dma_start(out=s01_d.ap(), in_=s01[:])

    nc.compile()
    return nc


def _core_rows(c):
    # column m = 128*f + p  ->  global row 512*(2f + (p>=64)) + 64*c + (p%64)
    f = np.arange(FT)[:, None]
    p = np.arange(128)[None, :]
    q = 2 * f + (p >= 64)
    return (QBLK * q + HALF * c + (p % 64)).reshape(-1)


def kernel(feat, center, labels):
    feat = np.asarray(feat, np.float32)
    center = np.asarray(center, np.float32)
    labels = np.asarray(labels).astype(np.int64)

    cf = feat - center                                   # [N, D] fp32
    sq64 = np.sum(cf.astype(np.float64) ** 2, axis=1)
    sq32 = sq64.astype(np.float32)
    cfb = cf.astype(BF16)
    sqh = sq32.astype(BF16)
    sql = (sq32 - sqh.astype(np.float32)).astype(BF16)

    # shared rhs A [KPAD, N] -> dram layout [128, NB, KCH, 512]
    A = np.zeros((KPAD, N), BF16)
    A[:D] = cfb.T
    A[D] = sqh
    A[D + 1] = sql
    A[D + 2] = np.ones(N, BF16)
    A[D + 3] = np.ones(N, BF16)
    a_dev = np.ascontiguousarray(
        A.reshape(KCH, 128, NB, 512).transpose(1, 2, 0, 3))

    oha = np.zeros((128, N), BF16)
    oh = (labels[None, :] == np.arange(64)[:, None])
    oha[:64] = oh.astype(BF16)
    oha[64] = np.ones(N, BF16)

    if "nc" not in _prog_cache:
        _prog_cache["nc"] = _build_program()
    nc = _prog_cache["nc"]

    in_maps = []
    rows_all = []
    for c in range(NCORES):
        rows = _core_rows(c)
        rows_all.append(rows)
        R = np.zeros((KPAD, 512), BF16)
        R[:D] = (-2.0 * cfb[rows].astype(np.float32)).astype(BF16).T
        R[D] = np.ones(512, BF16)
        R[D + 1] = np.ones(512, BF16)
        R[D + 2] = sqh[rows]
        R[D + 3] = sql[rows]
        rm2_dev = np.ascontiguousarray(
            R.reshape(KCH, 128, 512).transpose(1, 0, 2))

        ohm = np.zeros((128, 512), BF16)
        ohm[:64] = (-1000.0 * (labels[rows][None, :]
                    == np.arange(64)[:, None])).astype(BF16)

        m = np.zeros((128, NB, 512), BF16)
        jg = np.arange(512)
        for b in range(NB):
            ig = rows[128 * (b // 2):128 * (b // 2) + 128]
            m[:, b, :] = (-1000.0 * ((512 * b + jg)[None, :] <= ig[:, None])).astype(BF16)

        in_maps.append({"a": a_dev, "rm2": rm2_dev, "oha": oha,
                        "ohm": ohm, "masks": m})

    global _last_in_maps
    _last_in_maps = in_maps
    res = run_bass_kernel_spmd(nc, in_maps, list(range(NCORES)))

    S0 = np.zeros(N, np.float32)
    S1 = np.zeros(N, np.float32)
    for c in range(NCORES):
        s01 = np.asarray(res.results[c]["s01"], np.float32)[:, :8]
        S0[rows_all[c]] = s01[:, :FT].T.reshape(-1)
        S1[rows_all[c]] = s01[:, FT:].T.reshape(-1)

    loss_an = (np.float32(5.0) * S0 + S1) / (S0 + np.float32(1e-5))
    ranked = np.mean(loss_an, dtype=np.float32)

    ac = np.sqrt(np.clip(sq64, 1e-12, None))
    under = np.sum(np.where(ac < 3.0, 3.0 - ac, 0.0))
    beyond = np.sum(np.where(ac > 5.0, ac - 5.0, 0.0))
    annulus = np.float32((under + beyond) / N)

    return np.array(ranked + annulus, dtype=np.float32)



# revision 2
# speedup vs baseline: 4.6983x; 4.6983x over previous
"""Trainium2 Bass kernel for nn_ClusterLoss (N=4096, D=2048).

Math (constants ALPHA=6, BETA=2, ANN_R=3, ANN_RR=5, TVAL=1, EPS=1e-5):
  dm = 1 - dist <= 1 < BETA  =>  loss_ap == 0 identically.
  dm < ALPHA always          =>  an_mask == neg (upper-tri & label mismatch).
  loss_an_i = sum_j (5+u_ij) e^(5+u_ij) / (sum_j e^(5+u_ij) + EPS),  u = dist.
Device computes per-row S0 = sum w and S1 = sum u*w with w = e^(u+5) masked;
host does the division, mean, and the annulus term (O(N) work).

This environment measures the dispatch wall-clock (NTFF profiling is
unavailable under the axon tunnel), which is dominated by host->device input
transfer (~24ms/MB through the bass custom-call path on top of a ~0.18s
fixed round-trip). So the design minimizes shipped bytes: one core gets the
whole problem as a single fp8 feature matrix (~8.4MB) plus ~0.9MB of
auxiliaries, instead of the 8x-replicated bf16 inputs (~177MB) the previous
version shipped. On-device compute (~0.5ms) is invisible next to the fixed
dispatch cost, so single-core is as fast as 8-way SPMD here and much simpler.

Device algorithm, per [128,512] tile (m-tile x n-block), 144 upper tiles:
  P    = sum_k (s*cf_i_k)(s*cf_j_k)  [fp8 matmul, 16 K-chunks]
       + 1*hj + 1*lj + hi*1 + li*1   [bf16 K=4 matmul, h+l = -s^2/2*sq]
       => P = -s^2/2 * d2_ij  exactly (s=16 absorbed at the sqrt below)
  u    = Sqrt(P * (-2/s^2))          [ACT, negative scale folds the -2/s^2]
  nm   = -192*same_label             [fp8 K=64 one-hot matmul]
  u3   = u + nm (+ tri mask -192*(j<=i) on diagonal tiles)
  e    = Exp(u3 + 5) -> accum S0; S1 = reduce(u3 * e)
Masked entries give exp(u-187+5) -> 0 exactly in fp32, so masked terms drop
out of both sums. Host: loss = mean((5*S0+S1)/(S0+1e-5)) + annulus.
"""

import sys

sys.path.insert(0, "/opt/trn_rl_repo")

import numpy as np
import ml_dtypes

import concourse.bass as bass
import concourse.mybir as mybir
import concourse.tile as tile
from concourse import bacc
from concourse.bass_utils import run_bass_kernel_spmd

BF16 = ml_dtypes.bfloat16
FP8 = ml_dtypes.float8_e4m3
N, D = 4096, 2048
KCH = 16            # 2048 / 128 K-chunks for the feature matmul
NMT = 32            # 128-row m-tiles
NB = 8              # 512-col n-blocks
S = 16.0            # fp8 scale on cf; absorbed by the Sqrt activation scale
S2 = S * S
NEG = -192.0        # mask kill value (exact in fp8/bf16; exp(u+5-192) -> 0)
MIXED_GROUP = True  # bf16 aug matmul accumulates into the fp8 PSUM group

_prog_cache = {}


def _build_program():
    nc = bacc.Bacc("TRN2", target_bir_lowering=False, debug=False,
                   num_devices=1)

    # const AP for the Exp bias (+5.0), registered in the preamble like
    # Bass.__init__ does for 0.0/1.0
    t5 = nc.alloc_sbuf_tensor("const-float32-5.0", [128, 1], mybir.dt.float32)
    nc.gpsimd.memset(t5.ap(), 5.0)
    nc.const_aps.aps[(mybir.dt.float32, 5.0)] = t5.ap()
    nc.all_engine_barrier()

    f8 = mybir.dt.float8e4
    bf16 = mybir.dt.bfloat16
    fp32 = mybir.dt.float32

    x_d = nc.dram_tensor("x", [128, KCH, N], f8, kind="ExternalInput")
    oh_d = nc.dram_tensor("oh", [64, N], f8, kind="ExternalInput")
    ohm_d = nc.dram_tensor("ohm", [64, N], f8, kind="ExternalInput")
    augl_d = nc.dram_tensor("augl", [4, N], bf16, kind="ExternalInput")
    augr_d = nc.dram_tensor("augr", [4, N], bf16, kind="ExternalInput")
    tm_d = nc.dram_tensor("tm", [128, 4, 512], bf16, kind="ExternalInput")
    s01_d = nc.dram_tensor("s01", [128, 64], fp32, kind="ExternalOutput")

    with tile.TileContext(nc) as tc:
        with (
            tc.tile_pool(name="big", bufs=1) as big,
            tc.tile_pool(name="acc", bufs=1) as accp,
            tc.tile_pool(name="work", bufs=4) as work,
            tc.tile_pool(name="psum", bufs=3, space="PSUM") as psum,
        ):
            X = big.tile([128, KCH, N], f8)
            nc.sync.dma_start(out=X[:], in_=x_d.ap())
            oh = big.tile([64, N], f8)
            nc.sync.dma_start(out=oh[:], in_=oh_d.ap())
            ohm = big.tile([64, N], f8)
            nc.sync.dma_start(out=ohm[:], in_=ohm_d.ap())
            augl = big.tile([4, N], bf16)
            nc.sync.dma_start(out=augl[:], in_=augl_d.ap())
            augr = big.tile([4, N], bf16)
            nc.sync.dma_start(out=augr[:], in_=augr_d.ap())
            tm = big.tile([128, 4, 512], bf16)
            nc.sync.dma_start(out=tm[:], in_=tm_d.ap())

            s0a = accp.tile([128, NMT, NB], fp32, tag="s0a", name="s0a")
            s1a = accp.tile([128, NMT, NB], fp32, tag="s1a", name="s1a")

            for m in range(NMT):
                q = m // 4
                ml, mh = 128 * m, 128 * m + 128
                for b in range(q, NB):
                    bl, bh = 512 * b, 512 * b + 512
                    P = psum.tile([128, 512], fp32, tag="P")
                    for k in range(KCH):
                        nc.tensor.matmul(P[:], X[:, k, ml:mh], X[:, k, bl:bh],
                                         start=(k == 0), stop=False)
                    if MIXED_GROUP:
                        nc.tensor.matmul(P[:], augl[:, ml:mh], augr[:, bl:bh],
                                         start=False, stop=True)
                        Psrc = P
                    else:
                        nc.tensor.matmul(P[:], X[:, 0, ml:mh], X[:, 0, bl:bh],
                                         start=False, stop=True)
                        P2 = psum.tile([128, 512], fp32, tag="P2")
                        nc.tensor.matmul(P2[:], augl[:, ml:mh],
                                         augr[:, bl:bh], start=True, stop=True)
                        Ps = work.tile([128, 512], fp32, tag="Ps")
                        nc.vector.tensor_add(Ps[:], P[:], P2[:])
                        Psrc = Ps
                    nm = psum.tile([128, 512], fp32, tag="nm")
                    nc.tensor.matmul(nm[:], ohm[:, ml:mh], oh[:, bl:bh],
                                     start=True, stop=True)
                    # clamp P <= -eps so d2 = -2P/s^2 >= eps' (fp8 noise can
                    # push near-diagonal d2 slightly negative -> NaN sqrt)
                    Pc = work.tile([128, 512], fp32, tag="Pc")
                    nc.vector.tensor_scalar_min(Pc[:], Psrc[:], -1e-6)
                    u = work.tile([128, 512], fp32, tag="u")
                    nc.scalar.activation(u[:], Pc[:],
                                         mybir.ActivationFunctionType.Sqrt,
                                         scale=-2.0 / S2)
                    u2 = work.tile([128, 512], fp32, tag="u2")
                    nc.vector.tensor_add(u2[:], u[:], nm[:])
                    if b == q:
                        u3 = work.tile([128, 512], fp32, tag="u3")
                        nc.vector.tensor_add(u3[:], u2[:], tm[:, m % 4])
                    else:
                        u3 = u2
                    e = work.tile([128, 512], bf16, tag="e")
                    nc.scalar.activation(e[:], u3[:],
                                         mybir.ActivationFunctionType.Exp,
                                         bias=5.0, scale=1.0,
                                         accum_out=s0a[:, m, b:b + 1])
                    pm = work.tile([128, 512], bf16, tag="pm")
                    nc.vector.tensor_mul(pm[:], u3[:], e[:])
                    nc.vector.reduce_sum(out=s1a[:, m, b:b + 1], in_=pm[:],
                                         axis=mybir.AxisListType.X)

            s01 = accp.tile([128, 64], fp32, tag="s01", name="s01")
            for m in range(NMT):
                q = m // 4
                nc.vector.reduce_sum(out=s01[:, m:m + 1],
                                     in_=s0a[:, m, q:NB],
                                     axis=mybir.AxisListType.X)
                nc.vector.reduce_sum(out=s01[:, 32 + m:33 + m],
                                     in_=s1a[:, m, q:NB],
                                     axis=mybir.AxisListType.X)
            nc.sync.dma_start(out=s01_d.ap(), in_=s01[:])

    nc.compile()
    return nc


def kernel(feat, center, labels):
    feat = np.asarray(feat, np.float32)
    center = np.asarray(center, np.float32)
    labels = np.asarray(labels).astype(np.int64)

    cf = feat - center                                   # [N, D] fp32
    sq64 = np.sum(cf.astype(np.float64) ** 2, axis=1)
    sq32 = sq64.astype(np.float32)

    # X[p, k, j] = s * cf[j, 128k + p]  (fp8), the shared Gram operand
    scfT = (S * cf).T.astype(FP8)                        # [D, N]
    x_dev = np.ascontiguousarray(
        scfT.reshape(KCH, 128, N).transpose(1, 0, 2))

    oh = (labels[None, :] == np.arange(64)[:, None]).astype(FP8)    # [64, N]
    ohm = (NEG * oh.astype(np.float32)).astype(FP8)

    v = (-0.5 * S2) * sq32                               # [N] fp32
    h = v.astype(BF16)
    l = (v - h.astype(np.float32)).astype(BF16)
    ones = np.ones(N, BF16)
    augl = np.ascontiguousarray(np.stack([ones, ones, h, l]))    # [4, N]
    augr = np.ascontiguousarray(np.stack([h, l, ones, ones]))    # [4, N]

    p_i = np.arange(128)[:, None, None]
    r_i = np.arange(4)[None, :, None]
    j_i = np.arange(512)[None, None, :]
    tm = np.where(j_i <= 128 * r_i + p_i, np.float32(NEG),
                  np.float32(0.0)).astype(BF16)          # [128, 4, 512]

    if "nc" not in _prog_cache:
        _prog_cache["nc"] = _build_program()
    nc = _prog_cache["nc"]

    in_maps = [{"x": x_dev, "oh": oh, "ohm": ohm,
                "augl": augl, "augr": augr, "tm": tm}]
    global _last_in_maps
    _last_in_maps = in_maps
    res = run_bass_kernel_spmd(nc, in_maps, [0])

    s01 = np.asarray(res.results[0]["s01"], np.float32)
    S0 = s01[:, :32].T.reshape(N).copy()
    S1 = s01[:, 32:].T.reshape(N).copy()

    loss_an = (np.float32(5.0) * S0 + S1) / (S0 + np.float32(1e-5))
    ranked = np.mean(loss_an, dtype=np.float32)

    ac = np.sqrt(np.clip(sq64, 1e-12, None))
    under = np.sum(np.where(ac < 3.0, 3.0 - ac, 0.0))
    beyond = np.sum(np.where(ac > 5.0, ac - 5.0, 0.0))
    annulus = np.float32((under + beyond) / N)

    return np.array(ranked + annulus, dtype=np.float32)


# revision 4
# speedup vs baseline: 5.2983x; 1.1277x over previous
"""Trainium2 Bass kernel for nn_ClusterLoss (N=4096, D=2048).

Math (constants ALPHA=6, BETA=2, ANN_R=3, ANN_RR=5, TVAL=1, EPS=1e-5):
  dm = 1 - dist <= 1 < BETA  =>  loss_ap == 0 identically.
  dm < ALPHA always          =>  an_mask == neg (upper-tri & label mismatch).
  loss_an_i = sum_j (5+u_ij) e^(5+u_ij) / (sum_j e^(5+u_ij) + EPS),  u = dist.
Device computes per-row S0 = sum w and S1 = sum u*w with w = e^(u+5) masked;
host does the division, mean, and the annulus term (O(N) work).

This environment measures the dispatch wall-clock (NTFF profiling is
unavailable under the axon tunnel), which is dominated by host->device input
transfer (~24ms/MB through the bass custom-call path on top of a ~0.18s
fixed round-trip). So the design minimizes shipped bytes: one core gets the
whole problem as a single fp8 feature matrix (~8.4MB) plus ~0.9MB of
auxiliaries, instead of the 8x-replicated bf16 inputs (~177MB) the previous
version shipped. On-device compute (~0.5ms) is invisible next to the fixed
dispatch cost, so single-core is as fast as 8-way SPMD here and much simpler.

Device algorithm, per [128,512] tile (m-tile x n-block), 144 upper tiles:
  P    = sum_k (s*cf_i_k)(s*cf_j_k)  [fp8 matmul, 16 K-chunks]
       + 1*hj + 1*lj + hi*1 + li*1   [bf16 K=4 matmul, h+l = -s^2/2*sq]
       => P = -s^2/2 * d2_ij  exactly (s=16 absorbed at the sqrt below)
  u    = Sqrt(P * (-2/s^2))          [ACT, negative scale folds the -2/s^2]
  nm   = -192*same_label             [fp8 K=64 one-hot matmul]
  u3   = u + nm (+ tri mask -192*(j<=i) on diagonal tiles)
  e    = Exp(u3 + 5) -> accum S0; S1 = reduce(u3 * e)
Masked entries give exp(u-187+5) -> 0 exactly in fp32, so masked terms drop
out of both sums. Host: loss = mean((5*S0+S1)/(S0+1e-5)) + annulus.
"""

import sys

sys.path.insert(0, "/opt/trn_rl_repo")

import numpy as np
import ml_dtypes

import concourse.bass as bass
import concourse.mybir as mybir
import concourse.tile as tile
from concourse import bacc
from concourse.bass_utils import run_bass_kernel_spmd

BF16 = ml_dtypes.bfloat16
FP8 = ml_dtypes.float8_e4m3
N, D = 4096, 2048
KCH = 16            # 2048 / 128 K-chunks for the feature matmul
NMT = 32            # 128-row m-tiles
NB = 8              # 512-col n-blocks
S = 16.0            # fp8 scale on cf; absorbed by the Sqrt activation scale
S2 = S * S
NEG = -192.0        # mask kill value (exact in fp8/bf16; exp(u+5-192) -> 0)
GRP = 48            # tiles per sqrt/exp phase group (2 ACT table loads each)

_prog_cache = {}


def _build_program():
    nc = bacc.Bacc("TRN2", target_bir_lowering=False, debug=False,
                   num_devices=1)

    # const AP for the Exp bias (+5.0), registered in the preamble like
    # Bass.__init__ does for 0.0/1.0
    t5 = nc.alloc_sbuf_tensor("const-float32-5.0", [128, 1], mybir.dt.float32)
    nc.gpsimd.memset(t5.ap(), 5.0)
    nc.const_aps.aps[(mybir.dt.float32, 5.0)] = t5.ap()
    nc.all_engine_barrier()

    f8 = mybir.dt.float8e4
    bf16 = mybir.dt.bfloat16
    fp32 = mybir.dt.float32

    x_d = nc.dram_tensor("x", [128, KCH, N], f8, kind="ExternalInput")
    oh_d = nc.dram_tensor("oh", [64, N], f8, kind="ExternalInput")
    ohm_d = nc.dram_tensor("ohm", [64, N], f8, kind="ExternalInput")
    augl_d = nc.dram_tensor("augl", [4, N], bf16, kind="ExternalInput")
    augr_d = nc.dram_tensor("augr", [4, N], bf16, kind="ExternalInput")
    tm_d = nc.dram_tensor("tm", [128, 4, 512], bf16, kind="ExternalInput")
    s01_d = nc.dram_tensor("s01", [128, 64], fp32, kind="ExternalOutput")

    with tile.TileContext(nc) as tc:
        with (
            tc.tile_pool(name="big", bufs=1) as big,
            tc.tile_pool(name="acc", bufs=1) as accp,
            tc.tile_pool(name="work", bufs=4) as work,
            tc.tile_pool(name="upool", bufs=GRP) as upool,
            tc.tile_pool(name="psum", bufs=3, space="PSUM") as psum,
        ):
            X = big.tile([128, KCH, N], f8)
            nc.sync.dma_start(out=X[:], in_=x_d.ap())
            oh = big.tile([64, N], f8)
            nc.sync.dma_start(out=oh[:], in_=oh_d.ap())
            ohm = big.tile([64, N], f8)
            nc.sync.dma_start(out=ohm[:], in_=ohm_d.ap())
            augl = big.tile([4, N], bf16)
            nc.sync.dma_start(out=augl[:], in_=augl_d.ap())
            augr = big.tile([4, N], bf16)
            nc.sync.dma_start(out=augr[:], in_=augr_d.ap())
            tm = big.tile([128, 4, 512], bf16)
            nc.sync.dma_start(out=tm[:], in_=tm_d.ap())

            s0a = accp.tile([128, NMT, NB], fp32, tag="s0a", name="s0a")
            s1a = accp.tile([128, NMT, NB], fp32, tag="s1a", name="s1a")

            # sqrt and exp live in different ACT LUT sets; interleaving them
            # per tile forces an ~1ms table reload per switch (288 switches
            # = ~0.35s). Process tiles in groups: G sqrts then G exps, with
            # the u tiles buffered in SBUF as bf16 -> 2 loads per group.
            tiles = [(m, b) for m in range(NMT) for b in range(m // 4, NB)]
            for g0 in range(0, len(tiles), GRP):
                grp = tiles[g0:g0 + GRP]
                us = []
                for m, b in grp:
                    ml, mh = 128 * m, 128 * m + 128
                    bl, bh = 512 * b, 512 * b + 512
                    P = psum.tile([128, 512], fp32, tag="P")
                    for k in range(KCH):
                        nc.tensor.matmul(P[:], X[:, k, ml:mh], X[:, k, bl:bh],
                                         start=(k == 0), stop=False)
                    nc.tensor.matmul(P[:], augl[:, ml:mh], augr[:, bl:bh],
                                     start=False, stop=True)
                    # clamp P <= -eps so d2 = -2P/s^2 >= eps' (fp8 noise can
                    # push near-diagonal d2 slightly negative -> NaN sqrt)
                    Pc = work.tile([128, 512], fp32, tag="Pc")
                    nc.vector.tensor_scalar_min(Pc[:], P[:], -1e-6)
                    u = upool.tile([128, 512], bf16, tag="u")
                    nc.scalar.activation(u[:], Pc[:],
                                         mybir.ActivationFunctionType.Sqrt,
                                         scale=-2.0 / S2)
                    us.append(u)
                for (m, b), u in zip(grp, us):
                    ml, mh = 128 * m, 128 * m + 128
                    bl, bh = 512 * b, 512 * b + 512
                    nm = psum.tile([128, 512], fp32, tag="nm")
                    nc.tensor.matmul(nm[:], ohm[:, ml:mh], oh[:, bl:bh],
                                     start=True, stop=True)
                    u2 = work.tile([128, 512], fp32, tag="u2")
                    nc.vector.tensor_add(u2[:], u[:], nm[:])
                    if b == m // 4:
                        u3 = work.tile([128, 512], fp32, tag="u3")
                        nc.vector.tensor_add(u3[:], u2[:], tm[:, m % 4])
                    else:
                        u3 = u2
                    e = work.tile([128, 512], bf16, tag="e")
                    nc.scalar.activation(e[:], u3[:],
                                         mybir.ActivationFunctionType.Exp,
                                         bias=5.0, scale=1.0,
                                         accum_out=s0a[:, m, b:b + 1])
                    pm = work.tile([128, 512], bf16, tag="pm")
                    nc.vector.tensor_mul(pm[:], u3[:], e[:])
                    nc.vector.reduce_sum(out=s1a[:, m, b:b + 1], in_=pm[:],
                                         axis=mybir.AxisListType.X)

            s01 = accp.tile([128, 64], fp32, tag="s01", name="s01")
            for m in range(NMT):
                q = m // 4
                nc.vector.reduce_sum(out=s01[:, m:m + 1],
                                     in_=s0a[:, m, q:NB],
                                     axis=mybir.AxisListType.X)
                nc.vector.reduce_sum(out=s01[:, 32 + m:33 + m],
                                     in_=s1a[:, m, q:NB],
                                     axis=mybir.AxisListType.X)
            nc.sync.dma_start(out=s01_d.ap(), in_=s01[:])

    nc.compile()
    return nc


def kernel(feat, center, labels):
    feat = np.asarray(feat, np.float32)
    center = np.asarray(center, np.float32)
    labels = np.asarray(labels).astype(np.int64)

    cf = feat - center                                   # [N, D] fp32
    sq64 = np.sum(cf.astype(np.float64) ** 2, axis=1)
    sq32 = sq64.astype(np.float32)

    # X[p, k, j] = s * cf[j, 128k + p]  (fp8), the shared Gram operand
    scfT = (S * cf).T.astype(FP8)                        # [D, N]
    x_dev = np.ascontiguousarray(
        scfT.reshape(KCH, 128, N).transpose(1, 0, 2))

    oh = (labels[None, :] == np.arange(64)[:, None]).astype(FP8)    # [64, N]
    ohm = (NEG * oh.astype(np.float32)).astype(FP8)

    v = (-0.5 * S2) * sq32                               # [N] fp32
    h = v.astype(BF16)
    l = (v - h.astype(np.float32)).astype(BF16)
    ones = np.ones(N, BF16)
    augl = np.ascontiguousarray(np.stack([ones, ones, h, l]))    # [4, N]
    augr = np.ascontiguousarray(np.stack([h, l, ones, ones]))    # [4, N]

    p_i = np.arange(128)[:, None, None]
    r_i = np.arange(4)[None, :, None]
    j_i = np.arange(512)[None, None, :]
    tm = np.where(j_i <= 128 * r_i + p_i, np.float32(NEG),
                  np.float32(0.0)).astype(BF16)          # [128, 4, 512]

    if "nc" not in _prog_cache:
        _prog_cache["nc"] = _build_program()
    nc = _prog_cache["nc"]

    in_maps = [{"x": x_dev, "oh": oh, "ohm": ohm,
                "augl": augl, "augr": augr, "tm": tm}]
    global _last_in_maps
    _last_in_maps = in_maps
    res = run_bass_kernel_spmd(nc, in_maps, [0])

    s01 = np.asarray(res.results[0]["s01"], np.float32)
    S0 = s01[:, :32].T.reshape(N).copy()
    S1 = s01[:, 32:].T.reshape(N).copy()

    loss_an = (np.float32(5.0) * S0 + S1) / (S0 + np.float32(1e-5))
    ranked = np.mean(loss_an, dtype=np.float32)

    ac = np.sqrt(np.clip(sq64, 1e-12, None))
    under = np.sum(np.where(ac < 3.0, 3.0 - ac, 0.0))
    beyond = np.sum(np.where(ac > 5.0, ac - 5.0, 0.0))
    annulus = np.float32((under + beyond) / N)

    return np.array(ranked + annulus, dtype=np.float32)


# revision 8
# speedup vs baseline: 8.2387x; 1.5550x over previous
"""Trainium2 Bass kernel for nn_ClusterLoss (N=4096, D=2048, 8 NeuronCores).

Math (constants ALPHA=6, BETA=2, ANN_R=3, ANN_RR=5, TVAL=1, EPS=1e-5):
  dm = 1 - dist <= 1 < BETA  =>  loss_ap == 0 identically.
  dm < ALPHA always          =>  an_mask == neg (upper-tri & label mismatch).
  loss_an_i = sum_j (5+u_ij) e^(5+u_ij) / (sum_j e^(5+u_ij) + EPS),  u = dist.
Device computes per-row S0 = sum w and S1 = sum u*w with w = e^(u+5) masked;
host does the division, mean, and the annulus term (O(N) work).

Perf model for this environment (axon tunnel, no NTFF profiling): the
measured "HW exec time" is the dispatch wall-clock =
  ~0.18s round-trip + ~18ms/MB host->device input + device exec
where device exec costs ~19us/matmul-instruction + ~3ms/GMAC (and the 8
per-core NEFFs run in parallel; a device-side AllGather of the full 8.4MB
feature matrix costs only ~40ms). So:
  - ship minimal bytes: each core gets only its 1/8 column shard of the fp8
    feature matrix (1.05MB) + small aux; the full matrix is reassembled
    on-device via AllGather (total ~9MB vs 177MB replicated bf16 originally),
  - split compute 8 ways: core c owns global rows [512c, 512c+512).
SPMD uniformity: every core runs the identical program over all 32
(m_local, block) tiles; sub-diagonal blocks and the diagonal triangle are
masked via per-core gate vectors (Z = -192*[b<c], G = [b==c]) folded into
scalar_tensor_tensor ops, and the triangular masks are built on-device from
an iota (zero bytes shipped).

Per [128,512] tile: P = -s^2/2*d2 via fp8 Gram matmul (16 K-chunks) + bf16
K=4 aug matmul (hi/lo split of -s^2/2*sq rows); u = Sqrt(-2/s^2 * P) (scale
folds the constants); nm = -192*same_label via fp8 one-hot matmul;
u3 = (u + Z[b]) + nm + G[b]*tri; e = Exp(u3+5) -> accum S0; S1 = reduce(u3*e).
Masked entries underflow to exactly 0. sqrt/exp sit in different ACT LUT
sets, so all 32 sqrts run before all 32 exps (2 table loads, not 64).
"""

import sys

sys.path.insert(0, "/opt/trn_rl_repo")

import numpy as np
import ml_dtypes

import concourse.bass as bass
import concourse.mybir as mybir
import concourse.tile as tile
from concourse import bacc
from concourse.bass_utils import run_bass_kernel_spmd

BF16 = ml_dtypes.bfloat16
FP8 = ml_dtypes.float8_e4m3
N, D, NC = 4096, 2048, 8
KCH = 16            # 2048 / 128 K-chunks for the feature matmul
MLT = 4             # 128-row m-tiles per core (512-row shard)
NB = 8              # 512-col n-blocks (= AllGather rank blocks)
S = 16.0            # fp8 scale on cf; absorbed by the Sqrt activation scale
S2 = S * S
NEG = -192.0        # mask kill value (exact in fp8/bf16; exp(u+5-192) -> 0)

_prog_cache = {}


def _build_program():
    nc = bacc.Bacc("TRN2", target_bir_lowering=False, debug=False,
                   num_devices=NC)

    # const AP for the Exp bias (+5.0), registered in the preamble like
    # Bass.__init__ does for 0.0/1.0
    t5 = nc.alloc_sbuf_tensor("const-float32-5.0", [128, 1], mybir.dt.float32)
    nc.gpsimd.memset(t5.ap(), 5.0)
    nc.const_aps.aps[(mybir.dt.float32, 5.0)] = t5.ap()
    nc.all_engine_barrier()

    f8 = mybir.dt.float8e4
    bf16 = mybir.dt.bfloat16
    fp32 = mybir.dt.float32
    int32 = mybir.dt.int32
    Alu = mybir.AluOpType

    xs_d = nc.dram_tensor("xs", [128, KCH, 512], f8, kind="ExternalInput")
    oh_d = nc.dram_tensor("oh", [64, 512], f8, kind="ExternalInput")
    ohm_d = nc.dram_tensor("ohm", [64, 512], f8, kind="ExternalInput")
    augl_d = nc.dram_tensor("augl", [4, 512], bf16, kind="ExternalInput")
    augr_d = nc.dram_tensor("augr", [4, 512], bf16, kind="ExternalInput")
    zg_d = nc.dram_tensor("zg", [128, 16], fp32, kind="ExternalInput")
    s01_d = nc.dram_tensor("s01", [128, 8], fp32, kind="ExternalOutput")

    # AllGather outputs (Shared address space, rank-blocked)
    xg_d = nc.dram_tensor("xg", [NC, 128, KCH, 512], f8, addr_space="Shared")
    ohg_d = nc.dram_tensor("ohg", [NC, 64, 512], f8, addr_space="Shared")
    arg_d = nc.dram_tensor("arg", [NC, 4, 512], bf16, addr_space="Shared")

    with tile.TileContext(nc) as tc:
        with (
            tc.tile_pool(name="big", bufs=1) as big,
            tc.tile_pool(name="acc", bufs=1) as accp,
            tc.tile_pool(name="work", bufs=4) as work,
            tc.tile_pool(name="upool", bufs=MLT * NB) as upool,
            tc.tile_pool(name="psum", bufs=3, space="PSUM") as psum,
            tc.tile_pool(name="dram", bufs=1, space="DRAM") as dram,
        ):
            # bounce own shards into internal DRAM, all-gather, load to SBUF
            xs_b = dram.tile([128, KCH, 512], f8)
            nc.sync.dma_start(out=xs_b[:], in_=xs_d.ap())
            oh_b = dram.tile([64, 512], f8)
            nc.sync.dma_start(out=oh_b[:], in_=oh_d.ap())
            ar_b = dram.tile([4, 512], bf16)
            nc.sync.dma_start(out=ar_b[:], in_=augr_d.ap())
            nc.gpsimd.collective_compute(
                "AllGather", Alu.bypass, replica_groups=[list(range(NC))],
                ins=[xs_b[:]], outs=[xg_d.ap()])
            nc.gpsimd.collective_compute(
                "AllGather", Alu.bypass, replica_groups=[list(range(NC))],
                ins=[oh_b[:]], outs=[ohg_d.ap()])
            nc.gpsimd.collective_compute(
                "AllGather", Alu.bypass, replica_groups=[list(range(NC))],
                ins=[ar_b[:]], outs=[arg_d.ap()])

            Xg = big.tile([128, NB, KCH, 512], f8)
            ohg = big.tile([64, NB, 512], f8)
            arg = big.tile([4, NB, 512], bf16)
            for r in range(NB):
                nc.sync.dma_start(out=Xg[:, r], in_=xg_d.ap()[r])
                nc.sync.dma_start(out=ohg[:, r], in_=ohg_d.ap()[r])
                nc.sync.dma_start(out=arg[:, r], in_=arg_d.ap()[r])

            xs = big.tile([128, KCH, 512], f8)
            nc.sync.dma_start(out=xs[:], in_=xs_d.ap())
            ohm = big.tile([64, 512], f8)
            nc.sync.dma_start(out=ohm[:], in_=ohm_d.ap())
            augl = big.tile([4, 512], bf16)
            nc.sync.dma_start(out=augl[:], in_=augl_d.ap())
            zg = big.tile([128, 16], fp32)
            nc.sync.dma_start(out=zg[:], in_=zg_d.ap())

            # triangular masks built on device: W[p,j] = j - p, then
            # tm2[r] = -192 * (W <= 128r)   (mask j <= 128r + p)
            W = big.tile([128, 512], int32)
            nc.gpsimd.iota(W[:], pattern=[[1, 512]], base=0,
                           channel_multiplier=-1)
            tm2 = big.tile([128, MLT, 512], bf16)
            for r in range(MLT):
                nc.vector.tensor_scalar(tm2[:, r], W[:], float(128 * r), NEG,
                                        op0=Alu.is_le, op1=Alu.mult)

            s0a = accp.tile([128, MLT, NB], fp32, tag="s0a", name="s0a")
            s1a = accp.tile([128, MLT, NB], fp32, tag="s1a", name="s1a")

            # one phase group: 32 sqrts then 32 exps (2 ACT table loads)
            tiles = [(ml, b) for ml in range(MLT) for b in range(NB)]
            us = []
            for ml, b in tiles:
                lo, hi = 128 * ml, 128 * ml + 128
                P = psum.tile([128, 512], fp32, tag="P")
                for k in range(KCH):
                    nc.tensor.matmul(P[:], xs[:, k, lo:hi], Xg[:, b, k],
                                     start=(k == 0), stop=False)
                nc.tensor.matmul(P[:], augl[:, lo:hi], arg[:, b],
                                 start=False, stop=True)
                # clamp P <= -eps so d2 = -2P/s^2 >= eps' (fp8 noise can
                # push diagonal d2 slightly negative -> NaN sqrt)
                Pc = work.tile([128, 512], fp32, tag="Pc")
                nc.vector.tensor_scalar_min(Pc[:], P[:], -1e-6)
                u = upool.tile([128, 512], bf16, tag="u")
                nc.scalar.activation(u[:], Pc[:],
                                     mybir.ActivationFunctionType.Sqrt,
                                     scale=-2.0 / S2)
                us.append(u)
            for (ml, b), u in zip(tiles, us):
                lo, hi = 128 * ml, 128 * ml + 128
                nm = psum.tile([128, 512], fp32, tag="nm")
                nc.tensor.matmul(nm[:], ohm[:, lo:hi], ohg[:, b],
                                 start=True, stop=True)
                # u2 = (u + Z[b]) + nm;  u3 = tri*G[b] + u2
                u2 = work.tile([128, 512], fp32, tag="u2")
                nc.vector.scalar_tensor_tensor(u2[:], u[:], zg[:, b:b + 1],
                                               nm[:], op0=Alu.add,
                                               op1=Alu.add)
                u3 = work.tile([128, 512], fp32, tag="u3")
                nc.vector.scalar_tensor_tensor(u3[:], tm2[:, ml],
                                               zg[:, 8 + b:9 + b], u2[:],
                                               op0=Alu.mult, op1=Alu.add)
                e = work.tile([128, 512], bf16, tag="e")
                nc.scalar.activation(e[:], u3[:],
                                     mybir.ActivationFunctionType.Exp,
                                     bias=5.0, scale=1.0,
                                     accum_out=s0a[:, ml, b:b + 1])
                pm = work.tile([128, 512], bf16, tag="pm")
                nc.vector.tensor_mul(pm[:], u3[:], e[:])
                nc.vector.reduce_sum(out=s1a[:, ml, b:b + 1], in_=pm[:],
                                     axis=mybir.AxisListType.X)

            s01 = accp.tile([128, 8], fp32, tag="s01", name="s01")
            nc.vector.reduce_sum(out=s01[:, 0:4], in_=s0a[:],
                                 axis=mybir.AxisListType.X)
            nc.vector.reduce_sum(out=s01[:, 4:8], in_=s1a[:],
                                 axis=mybir.AxisListType.X)
            nc.sync.dma_start(out=s01_d.ap(), in_=s01[:])

    nc.compile()
    return nc


def kernel(feat, center, labels):
    feat = np.asarray(feat, np.float32)
    center = np.asarray(center, np.float32)
    labels = np.asarray(labels).astype(np.int64)

    cf = feat - center                                   # [N, D] fp32
    sq64 = np.sum(cf.astype(np.float64) ** 2, axis=1)
    sq32 = sq64.astype(np.float32)

    # X[p, k, j] = s * cf[j, 128k + p]  (fp8), the shared Gram operand
    scfT = (S * cf).T.astype(FP8)                        # [D, N]
    xg = np.ascontiguousarray(scfT.reshape(KCH, 128, N).transpose(1, 0, 2))

    ohf = (labels[None, :] == np.arange(64)[:, None]).astype(FP8)   # [64, N]
    ohmf = (NEG * ohf.astype(np.float32)).astype(FP8)

    v = (-0.5 * S2) * sq32                               # [N] fp32
    h = v.astype(BF16)
    l = (v - h.astype(np.float32)).astype(BF16)
    ones = np.ones(N, BF16)
    auglf = np.ascontiguousarray(np.stack([ones, ones, h, l]))   # [4, N]
    augrf = np.ascontiguousarray(np.stack([h, l, ones, ones]))   # [4, N]

    if "nc" not in _prog_cache:
        _prog_cache["nc"] = _build_program()
    nc = _prog_cache["nc"]

    in_maps = []
    for c in range(NC):
        sl = slice(512 * c, 512 * c + 512)
        zg = np.zeros((128, 16), np.float32)
        zg[:, :NC] = np.where(np.arange(NC)[None, :] < c, NEG, 0.0)
        zg[:, 8 + c] = 1.0
        in_maps.append({
            "xs": np.ascontiguousarray(xg[:, :, sl]),
            "oh": np.ascontiguousarray(ohf[:, sl]),
            "ohm": np.ascontiguousarray(ohmf[:, sl]),
            "augl": np.ascontiguousarray(auglf[:, sl]),
            "augr": np.ascontiguousarray(augrf[:, sl]),
            "zg": zg,
        })
    global _last_in_maps
    _last_in_maps = in_maps
    res = run_bass_kernel_spmd(nc, in_maps, list(range(NC)))

    S0 = np.zeros(N, np.float32)
    S1 = np.zeros(N, np.float32)
    for c in range(NC):
        s01 = np.asarray(res.results[c]["s01"], np.float32)   # [128, 8]
        S0[512 * c:512 * c + 512] = s01[:, 0:4].T.reshape(512)
        S1[512 * c:512 * c + 512] = s01[:, 4:8].T.reshape(512)

    loss_an = (np.float32(5.0) * S0 + S1) / (S0 + np.float32(1e-5))
    ranked = np.mean(loss_an, dtype=np.float32)

    ac = np.sqrt(np.clip(sq64, 1e-12, None))
    under = np.sum(np.where(ac < 3.0, 3.0 - ac, 0.0))
    beyond = np.sum(np.where(ac > 5.0, ac - 5.0, 0.0))
    annulus = np.float32((under + beyond) / N)

    return np.array(ranked + annulus, dtype=np.float32)


# revision 9
# speedup vs baseline: 8.6506x; 1.0500x over previous
"""Trainium2 Bass kernel for nn_ClusterLoss (N=4096, D=2048, 8 NeuronCores).

Math (constants ALPHA=6, BETA=2, ANN_R=3, ANN_RR=5, TVAL=1, EPS=1e-5):
  dm = 1 - dist <= 1 < BETA  =>  loss_ap == 0 identically.
  dm < ALPHA always          =>  an_mask == neg (upper-tri & label mismatch).
  loss_an_i = sum_j (5+u_ij) e^(5+u_ij) / (sum_j e^(5+u_ij) + EPS),  u = dist.
Device computes per-row S0 = sum w and S1 = sum u*w with w = e^(u+5) masked;
host does the division, mean, and the annulus term (O(N) work).

Perf model for this environment (axon tunnel, no NTFF profiling): the
measured "HW exec time" is the dispatch wall-clock =
  ~0.18s round-trip + ~18ms/MB host->device input + device exec
where device exec costs ~19us/matmul-instruction + ~3ms/GMAC (and the 8
per-core NEFFs run in parallel; a device-side AllGather of the full 8.4MB
feature matrix costs only ~40ms). So:
  - ship minimal bytes: each core gets only its 1/8 column shard of the fp8
    feature matrix (1.05MB) + small aux; the full matrix is reassembled
    on-device via AllGather (total ~9MB vs 177MB replicated bf16 originally),
  - split compute 8 ways: core c owns global rows [512c, 512c+512).
SPMD uniformity: every core runs the identical program over all 32
(m_local, block) tiles; sub-diagonal blocks and the diagonal triangle are
masked via per-core gate vectors (Z = -192*[b<c], G = [b==c]) folded into
scalar_tensor_tensor ops, and the triangular masks are built on-device from
an iota (zero bytes shipped).

Per [128,512] tile: P = -s^2/2*d2 via fp8 Gram matmul (16 K-chunks) + bf16
K=4 aug matmul (hi/lo split of -s^2/2*sq rows); u = Sqrt(-2/s^2 * P) (scale
folds the constants); nm = -192*same_label via fp8 one-hot matmul;
u3 = (u + Z[b]) + nm + G[b]*tri; e = Exp(u3+5) -> accum S0; S1 = reduce(u3*e).
Masked entries underflow to exactly 0. sqrt/exp sit in different ACT LUT
sets, so all 32 sqrts run before all 32 exps (2 table loads, not 64).
"""

import sys

sys.path.insert(0, "/opt/trn_rl_repo")

import numpy as np
import ml_dtypes

import concourse.bass as bass
import concourse.mybir as mybir
import concourse.tile as tile
from concourse import bacc
from concourse.bass_utils import run_bass_kernel_spmd

BF16 = ml_dtypes.bfloat16
FP8 = ml_dtypes.float8_e4m3
N, D, NC = 4096, 2048, 8
KCH = 16            # 2048 / 128 K-chunks for the feature matmul
MLT = 4             # 128-row m-tiles per core (512-row shard)
NB = 8              # 512-col n-blocks (= AllGather rank blocks)
S = 16.0            # fp8 scale on cf; absorbed by the Sqrt activation scale
S2 = S * S
NEG = -192.0        # mask kill value (exact in fp8/bf16; exp(u+5-192) -> 0)

_prog_cache = {}


def _build_program():
    nc = bacc.Bacc("TRN2", target_bir_lowering=False, debug=False,
                   num_devices=NC)

    # const AP for the Exp bias (+5.0), registered in the preamble like
    # Bass.__init__ does for 0.0/1.0
    t5 = nc.alloc_sbuf_tensor("const-float32-5.0", [128, 1], mybir.dt.float32)
    nc.gpsimd.memset(t5.ap(), 5.0)
    nc.const_aps.aps[(mybir.dt.float32, 5.0)] = t5.ap()
    nc.all_engine_barrier()

    f8 = mybir.dt.float8e4
    bf16 = mybir.dt.bfloat16
    fp32 = mybir.dt.float32
    int32 = mybir.dt.int32
    Alu = mybir.AluOpType

    xs_d = nc.dram_tensor("xs", [128, KCH, 512], f8, kind="ExternalInput")
    oh_d = nc.dram_tensor("oh", [64, 512], f8, kind="ExternalInput")
    augl_d = nc.dram_tensor("augl", [4, 512], bf16, kind="ExternalInput")
    augr_d = nc.dram_tensor("augr", [4, 512], bf16, kind="ExternalInput")
    zg_d = nc.dram_tensor("zg", [128, 16], fp32, kind="ExternalInput")
    s01_d = nc.dram_tensor("s01", [128, 8], fp32, kind="ExternalOutput")

    # AllGather outputs (Shared address space, rank-blocked)
    xg_d = nc.dram_tensor("xg", [NC, 128, KCH, 512], f8, addr_space="Shared")
    ohg_d = nc.dram_tensor("ohg", [NC, 64, 512], f8, addr_space="Shared")
    arg_d = nc.dram_tensor("arg", [NC, 4, 512], bf16, addr_space="Shared")

    with tile.TileContext(nc) as tc:
        with (
            tc.tile_pool(name="big", bufs=1) as big,
            tc.tile_pool(name="acc", bufs=1) as accp,
            tc.tile_pool(name="work", bufs=4) as work,
            tc.tile_pool(name="upool", bufs=MLT * NB) as upool,
            tc.tile_pool(name="psum", bufs=3, space="PSUM") as psum,
            tc.tile_pool(name="dram", bufs=1, space="DRAM") as dram,
        ):
            # bounce own shards into internal DRAM, all-gather, load to SBUF
            xs_b = dram.tile([128, KCH, 512], f8)
            nc.sync.dma_start(out=xs_b[:], in_=xs_d.ap())
            oh_b = dram.tile([64, 512], f8)
            nc.sync.dma_start(out=oh_b[:], in_=oh_d.ap())
            ar_b = dram.tile([4, 512], bf16)
            nc.sync.dma_start(out=ar_b[:], in_=augr_d.ap())
            nc.gpsimd.collective_compute(
                "AllGather", Alu.bypass, replica_groups=[list(range(NC))],
                ins=[xs_b[:]], outs=[xg_d.ap()])
            nc.gpsimd.collective_compute(
                "AllGather", Alu.bypass, replica_groups=[list(range(NC))],
                ins=[oh_b[:]], outs=[ohg_d.ap()])
            nc.gpsimd.collective_compute(
                "AllGather", Alu.bypass, replica_groups=[list(range(NC))],
                ins=[ar_b[:]], outs=[arg_d.ap()])

            Xg = big.tile([128, NB, KCH, 512], f8)
            ohg = big.tile([64, NB, 512], f8)
            arg = big.tile([4, NB, 512], bf16)
            for r in range(NB):
                nc.sync.dma_start(out=Xg[:, r], in_=xg_d.ap()[r])
                nc.sync.dma_start(out=ohg[:, r], in_=ohg_d.ap()[r])
                nc.sync.dma_start(out=arg[:, r], in_=arg_d.ap()[r])

            xs = big.tile([128, KCH, 512], f8)
            nc.sync.dma_start(out=xs[:], in_=xs_d.ap())
            oho = big.tile([64, 512], f8)
            nc.sync.dma_start(out=oho[:], in_=oh_d.ap())
            augl = big.tile([4, 512], bf16)
            nc.sync.dma_start(out=augl[:], in_=augl_d.ap())
            zg = big.tile([128, 16], fp32)
            nc.sync.dma_start(out=zg[:], in_=zg_d.ap())

            # triangular masks built on device: W[p,j] = j - p, then
            # tm2[r] = -192 * (W <= 128r)   (mask j <= 128r + p)
            W = big.tile([128, 512], int32)
            nc.gpsimd.iota(W[:], pattern=[[1, 512]], base=0,
                           channel_multiplier=-1)
            tm2 = big.tile([128, MLT, 512], bf16)
            for r in range(MLT):
                nc.vector.tensor_scalar(tm2[:, r], W[:], float(128 * r), NEG,
                                        op0=Alu.is_le, op1=Alu.mult)

            s0a = accp.tile([128, MLT, NB], fp32, tag="s0a", name="s0a")
            s1a = accp.tile([128, MLT, NB], fp32, tag="s1a", name="s1a")

            # one phase group: 32 sqrts then 32 exps (2 ACT table loads)
            tiles = [(ml, b) for ml in range(MLT) for b in range(NB)]
            us = []
            for ml, b in tiles:
                lo, hi = 128 * ml, 128 * ml + 128
                P = psum.tile([128, 512], fp32, tag="P")
                for k in range(0, KCH, 2):
                    nc.tensor.matmul(P[:], xs[:, k:k + 2, lo:hi],
                                     Xg[:, b, k:k + 2],
                                     start=(k == 0), stop=False,
                                     perf_mode=mybir.MatmulPerfMode.DoubleRow)
                nc.tensor.matmul(P[:], augl[:, lo:hi], arg[:, b],
                                 start=False, stop=True)
                # clamp P <= -eps so d2 = -2P/s^2 >= eps' (fp8 noise can
                # push diagonal d2 slightly negative -> NaN sqrt)
                Pc = work.tile([128, 512], fp32, tag="Pc")
                nc.vector.tensor_scalar_min(Pc[:], P[:], -1e-6)
                u = upool.tile([128, 512], bf16, tag="u")
                nc.scalar.activation(u[:], Pc[:],
                                     mybir.ActivationFunctionType.Sqrt,
                                     scale=-2.0 / S2)
                us.append(u)
            for (ml, b), u in zip(tiles, us):
                lo, hi = 128 * ml, 128 * ml + 128
                nm = psum.tile([128, 512], fp32, tag="nm")
                nc.tensor.matmul(nm[:], oho[:, lo:hi], ohg[:, b],
                                 start=True, stop=True)
                # u2 = -192*same + u;  u3 = tri*G[b] + u2;
                # the sub-diagonal block kill Z[b] rides the Exp bias
                # (5 + Z[b]): masked entries get e = 0, which also zeroes
                # their S1 contribution u3*e.
                u2 = work.tile([128, 512], fp32, tag="u2")
                nc.vector.scalar_tensor_tensor(u2[:], nm[:], NEG,
                                               u[:], op0=Alu.mult,
                                               op1=Alu.add)
                u3 = work.tile([128, 512], fp32, tag="u3")
                nc.vector.scalar_tensor_tensor(u3[:], tm2[:, ml],
                                               zg[:, 8 + b:9 + b], u2[:],
                                               op0=Alu.mult, op1=Alu.add)
                e = work.tile([128, 512], bf16, tag="e")
                nc.scalar.activation(e[:], u3[:],
                                     mybir.ActivationFunctionType.Exp,
                                     bias=zg[:, b:b + 1], scale=1.0,
                                     accum_out=s0a[:, ml, b:b + 1])
                pm = work.tile([128, 512], bf16, tag="pm")
                nc.vector.tensor_mul(pm[:], u3[:], e[:])
                nc.vector.reduce_sum(out=s1a[:, ml, b:b + 1], in_=pm[:],
                                     axis=mybir.AxisListType.X)

            s01 = accp.tile([128, 8], fp32, tag="s01", name="s01")
            nc.vector.reduce_sum(out=s01[:, 0:4], in_=s0a[:],
                                 axis=mybir.AxisListType.X)
            nc.vector.reduce_sum(out=s01[:, 4:8], in_=s1a[:],
                                 axis=mybir.AxisListType.X)
            nc.sync.dma_start(out=s01_d.ap(), in_=s01[:])

    nc.compile()
    return nc


def kernel(feat, center, labels):
    feat = np.asarray(feat, np.float32)
    center = np.asarray(center, np.float32)
    labels = np.asarray(labels).astype(np.int64)

    cf = feat - center                                   # [N, D] fp32
    sq64 = np.sum(cf.astype(np.float64) ** 2, axis=1)
    sq32 = sq64.astype(np.float32)

    # X[p, k, j] = s * cf[j, 128k + p]  (fp8), the shared Gram operand
    scfT = (S * cf).T.astype(FP8)                        # [D, N]
    xg = np.ascontiguousarray(scfT.reshape(KCH, 128, N).transpose(1, 0, 2))

    ohf = (labels[None, :] == np.arange(64)[:, None]).astype(FP8)   # [64, N]

    v = (-0.5 * S2) * sq32                               # [N] fp32
    h = v.astype(BF16)
    l = (v - h.astype(np.float32)).astype(BF16)
    ones = np.ones(N, BF16)
    auglf = np.ascontiguousarray(np.stack([ones, ones, h, l]))   # [4, N]
    augrf = np.ascontiguousarray(np.stack([h, l, ones, ones]))   # [4, N]

    if "nc" not in _prog_cache:
        _prog_cache["nc"] = _build_program()
    nc = _prog_cache["nc"]

    in_maps = []
    for c in range(NC):
        sl = slice(512 * c, 512 * c + 512)
        zg = np.zeros((128, 16), np.float32)
        zg[:, :NC] = 5.0 + np.where(np.arange(NC)[None, :] < c, NEG, 0.0)
        zg[:, 8 + c] = 1.0
        in_maps.append({
            "xs": np.ascontiguousarray(xg[:, :, sl]),
            "oh": np.ascontiguousarray(ohf[:, sl]),
            "augl": np.ascontiguousarray(auglf[:, sl]),
            "augr": np.ascontiguousarray(augrf[:, sl]),
            "zg": zg,
        })
    global _last_in_maps
    _last_in_maps = in_maps
    res = run_bass_kernel_spmd(nc, in_maps, list(range(NC)))

    S0 = np.zeros(N, np.float32)
    S1 = np.zeros(N, np.float32)
    for c in range(NC):
        s01 = np.asarray(res.results[c]["s01"], np.float32)   # [128, 8]
        S0[512 * c:512 * c + 512] = s01[:, 0:4].T.reshape(512)
        S1[512 * c:512 * c + 512] = s01[:, 4:8].T.reshape(512)

    loss_an = (np.float32(5.0) * S0 + S1) / (S0 + np.float32(1e-5))
    ranked = np.mean(loss_an, dtype=np.float32)

    ac = np.sqrt(np.clip(sq64, 1e-12, None))
    under = np.sum(np.where(ac < 3.0, 3.0 - ac, 0.0))
    beyond = np.sum(np.where(ac > 5.0, ac - 5.0, 0.0))
    annulus = np.float32((under + beyond) / N)

    return np.array(ranked + annulus, dtype=np.float32)
